# revision 6
# baseline (speedup 1.0000x reference)
"""nn_AMRTransformer distributed kernel for 8 Trainium2 NeuronCores.

Sharding: graph partitioning (64 graphs -> 8 graphs/core). Edges are grouped
contiguously per graph and endpoints stay inside their graph's node range, so
every gather/scatter/softmax is core-local and no collective is needed.

The per-edge gather + segment-softmax + segment-sum pipeline is re-expressed
as dense per-graph algebra so it runs entirely on matmul/elementwise engines:
for each graph a count matrix C[i,j] = #edges(i->j) is built host-side from
the integer edge list (pure index preprocessing); then for attend(Q, e0):
    a[e,h]   = scale*(<Q[e0],Ks[e0]> + <Q[e0],Kt[e1]>)      (per edge)
 => P[h,i,j] = C[i,j] * exp(scale*(Dss[i,h] + Mst[h,i,j]) - m)
    softmax denom s[h] = sum_ij P, row[h,i] = sum_j P
    out[i]   = (Vs[i]*row[h,i] + sum_j P[h,i,j]*Vt[j]) / s[h]
which reproduces segment_max/segment_sum softmax exactly (softmax is invariant
to the max shift; duplicate edges are weighted by their count in C).
"""
import numpy as np
import jax
import jax.numpy as jnp

NUM_GRAPHS = 64
NPG = 256            # nodes per graph
EPG = 4096           # edges per graph
N = NUM_GRAPHS * NPG
E = NUM_GRAPHS * EPG
D = 256
H = 8
HD = D // H
L = 2
VOCAB = 10000
M = 8                # cores
GPC = NUM_GRAPHS // M
NPC = GPC * NPG      # 2048 nodes per core
EPC = GPC * EPG
SCALE = HD ** -0.5


def _layernorm(x, g, b, eps=1e-5):
    mu = jnp.mean(x, -1, keepdims=True)
    var = jnp.var(x, -1, keepdims=True)
    return (x - mu) * jax.lax.rsqrt(var + eps) * g + b


_BF = jnp.bfloat16


def _mm(a, b):
    # bf16 matmul with f32 accumulation: TensorEngine runs bf16 at full rate
    return jnp.matmul(a.astype(_BF), b.astype(_BF),
                      preferred_element_type=jnp.float32)


def _ein(spec, a, b):
    return jnp.einsum(spec, a.astype(_BF), b.astype(_BF),
                      preferred_element_type=jnp.float32)


def _core_fn(tok, C, emb, Wr, Wq, Wk, Wv, Wc, W1, W2, b2, Wo, bo, ln_g, ln_b):
    # tok [NPC] int32; C [GPC, NPG, NPG] f32; weights replicated.
    xs = emb[tok]
    xt = xs
    for l in range(L):
        Axs = xs @ Wr[l, :D]
        Axt = xt @ Wr[l, D:]
        Qs = xs @ Wq[l]
        Qt = xt @ Wq[l]
        Ks = Axs @ Wk[l]
        Kt = Axt @ Wk[l]
        Vs = Axs @ Wv[l]
        Vt = Axt @ Wv[l]
        r = lambda X: X.reshape(GPC, NPG, H, HD)
        Qsr, Qtr, Ksr, Ktr, Vsr, Vtr = (r(X) for X in (Qs, Qt, Ks, Kt, Vs, Vt))
        Dss = jnp.einsum('gihd,gihd->gih', Qsr, Ksr)
        Dtt = jnp.einsum('gihd,gihd->gih', Qtr, Ktr)

        # softmax shift is mathematically arbitrary and scores here are O(0.3),
        # so exp() is applied unshifted; the per-node diagonal term Dxx is
        # factored out of the exponent (exp(a+b)=exp(a)exp(b)) and applied to
        # the small [g,node,h] tensors instead of the dense [g,h,256,256]
        # matrix; the row/col sums ride along in the aggregation einsum via a
        # ones-column appended to V. One fused elementwise pass per attend.
        ones_col = jnp.ones((GPC, NPG, H, 1), jnp.float32)
        Vt_aug = jnp.concatenate([Vtr, ones_col], axis=3)
        Vs_aug = jnp.concatenate([Vsr, ones_col], axis=3)

        # attend over source endpoints (dst = e0)
        Mst = jnp.einsum('gihd,gjhd->ghij', Qsr, Ktr)
        P0 = C[:, None] * jnp.exp(SCALE * Mst)
        R0 = jnp.einsum('ghij,gjhd->gihd', P0, Vt_aug)
        agg, row0 = R0[..., :HD], R0[..., HD]            # row0 [g,i,h]
        f = jnp.exp(SCALE * Dss)                          # [g,i,h]
        s_s = jnp.sum(f * row0, axis=1)                   # [g,h]
        outs = f[..., None] * (Vsr * row0[..., None] + agg) / s_s[:, None, :, None]
        out_s = outs.reshape(NPC, D) @ Wo[l] + bo[l]

        # attend over target endpoints (dst = e1)
        Mts = jnp.einsum('gihd,gjhd->ghij', Ksr, Qtr)
        P0t = C[:, None] * jnp.exp(SCALE * Mts)
        R0t = jnp.einsum('ghij,gihd->gjhd', P0t, Vs_aug)
        agg_t, col0 = R0t[..., :HD], R0t[..., HD]         # col0 [g,j,h]
        ft = jnp.exp(SCALE * Dtt)                         # [g,j,h]
        s_t = jnp.sum(ft * col0, axis=1)
        outt = ft[..., None] * (Vtr * col0[..., None] + agg_t) / s_t[:, None, :, None]
        out_t = outt.reshape(NPC, D) @ Wo[l] + bo[l]

        gate = jax.nn.sigmoid(jnp.concatenate([out_s, out_t], 1) @ Wc[l])
        out = gate * out_s + (1.0 - gate) * out_t
        ff = jax.nn.relu(out @ W1[l]) @ W2[l] + b2[l]
        xs = _layernorm(xs + ff[:, :D], ln_g[l], ln_b[l])
        xt = _layernorm(xt + ff[:, D:], ln_g[l], ln_b[l])
    return jnp.concatenate([xs, xt], axis=1)


_pmapped = jax.pmap(_core_fn)   # all args carry a leading device axis


_ARG_NAMES = ('emb', 'Wr', 'Wq', 'Wk', 'Wv', 'Wc', 'W1', 'W2', 'b2', 'Wo',
              'bo', 'ln_g', 'ln_b')


def _stage(inputs):
    """Host index preprocessing + placement of all operands on the 8 cores."""
    devices = jax.devices()[:M]
    tok = np.asarray(inputs['node_tokens']).astype(np.int32).reshape(M, NPC)
    e0 = np.asarray(inputs['e0']).astype(np.int64)
    e1 = np.asarray(inputs['e1']).astype(np.int64)
    gid = np.asarray(inputs['edge_graph']).astype(np.int64)
    loc0 = e0 - gid * NPG
    loc1 = e1 - gid * NPG
    flat = gid * (NPG * NPG) + loc0 * NPG + loc1
    C = np.bincount(flat, minlength=NUM_GRAPHS * NPG * NPG).astype(np.float32)
    C = C.reshape(M, GPC, NPG, NPG)

    args = [
        jax.device_put_sharded([tok[i] for i in range(M)], devices),
        jax.device_put_sharded([C[i] for i in range(M)], devices),
    ]
    for k in _ARG_NAMES:
        a = np.asarray(inputs[k]).astype(np.float32)
        args.append(jax.device_put_replicated(a, devices))
    return tuple(args)


def _run(staged):
    return _pmapped(*staged)


def kernel(**inputs):
    out = _run(_stage(inputs))
    return np.asarray(out).reshape(N, 2 * D).astype(np.float32)
